# revision 1
# baseline (speedup 1.0000x reference)
"""DepthNet cost-volume kernel for 8 Trainium2 NeuronCores.

Strategy: shard output rows (H=128 -> 16 rows/core, +1 halo row each side).
Per core, per (view, halo-row): compute homography sample coords on DVE,
gather all four bilinear corners with one 512B dma_gather descriptor per
sample from a host-built "row-pair zip" table, blend corners on DVE,
accumulate sum/sum-of-squares volumes, write the variance volume to DRAM,
then run the 3x3x3 conv as 144 PSUM-accumulated banded matmuls per output
row and a per-pixel softmax over depth. No cross-core communication.
"""
import numpy as np
import concourse.bass as bass
import concourse.tile as tile
from concourse import bacc, mybir
from concourse import library_config

F32 = mybir.dt.float32
I16 = mybir.dt.int16
OP = mybir.AluOpType
ACT = mybir.ActivationFunctionType

B, C, H, W, D, V = 1, 32, 128, 160, 64, 5
NCORES = 8
ROWS = H // NCORES          # output rows per core
RH = ROWS + 2               # with halo
NENT = (H + 1) * W + 3      # zip table rows (front pad + (H+1)*W + 2 back pad)
J = 80                      # gather slots per partition; chunk=(view,row)=128*J samples
NID = 128 * J               # 10240 = D*W
NQ = 16                     # chunks of the (c,d') contraction (32*64/128)
VT_P, VT_C, VT_R, VT_J = 128, C, RH, 80   # var_T dram layout [p=(xh,d), c, r, j]
EMAX = float(NENT - 2)      # max legal fetch row (clamp; row EMAX+1 exists)

_cache = {}


def _build_program(nrep=1):
    nc = bacc.Bacc("TRN2", target_bir_lowering=False, debug=False,
                   num_devices=NCORES, num_swdge_queues=4)
    tabs = [nc.dram_tensor(f"tab{v}", [NENT, 64], F32, kind="ExternalInput")
            for v in range(1, V)]
    f0s = nc.dram_tensor("f0slab", [RH, 2, C, 80], F32, kind="ExternalInput")
    coefa = nc.dram_tensor("coefa", [128, 12], F32, kind="ExternalInput")
    coefb = nc.dram_tensor("coefb", [128, 12 * RH], F32, kind="ExternalInput")
    xgh = nc.dram_tensor("xg", [128, J], F32, kind="ExternalInput")
    bandh = nc.dram_tensor("band", [9 * NQ, 128, 64], F32, kind="ExternalInput")
    identh = nc.dram_tensor("ident", [128, 128], F32, kind="ExternalInput")
    var_t = nc.dram_tensor("var_t", [VT_P, VT_C, VT_R, VT_J], F32)
    outh = nc.dram_tensor("out", [ROWS, W, D], F32, kind="ExternalOutput")

    with tile.TileContext(nc) as tc:
        _emit(tc, nc, tabs, f0s, coefa, coefb, xgh, bandh, identh, var_t, outh,
              nrep)
    # SWDGE queue must be consistent with Tile's DMASW sem rotation, which
    # walks pool DMAs in final scheduled order: sem k pairs with queue k%4.
    cnt = 0
    for blk in nc.m.functions[0].blocks:
        for inst in blk.instructions:
            if isinstance(inst, mybir.InstDMAGatherAnt):
                inst.queue_num = cnt % 4
                cnt += 1
    nc.compile()
    return nc


def _emit(tc, nc, tabs, f0s, coefa, coefb, xgh, bandh, identh, var_t, outh,
          nrep):
    import contextlib
    with contextlib.ExitStack() as ctx:
        const_p = ctx.enter_context(tc.tile_pool(name="const", bufs=1))
        libi = nc.gpsimd.load_library(library_config.mlp)
        tc._libi = libi

        ca = const_p.tile([128, 12], F32)
        nc.sync.dma_start(ca[:], coefa.ap())
        cb = const_p.tile([128, 12 * RH], F32)
        nc.sync.dma_start(cb[:], coefb.ap())
        xg = const_p.tile([128, J], F32)
        nc.sync.dma_start(xg[:], xgh.ap())

        import os
        stage = os.environ.get("K_STAGE", "all")
        for rep in range(nrep):
            if rep > 0:
                tc.strict_bb_all_engine_barrier()
            if stage in ("all", "main"):
                _emit_main(ctx, tc, nc, tabs, f0s, ca, cb, xg, var_t, rep)
                tc.strict_bb_all_engine_barrier()
            if stage in ("all", "conv"):
                _emit_conv(ctx, tc, nc, bandh, identh, var_t, outh, rep)


def _emit_main(ctx, tc, nc, tabs, f0s, ca, cb, xg, var_t, rep):
    """Gather + variance volume. Writes var_t[p=(xh,d), c, r, j]."""
    import contextlib
    import os
    sub = os.environ.get("K_SUB", "full")  # coords | gather | full
    with contextlib.ExitStack() as st:
        volp = st.enter_context(tc.tile_pool(name="vol", bufs=2))
        crd = st.enter_context(tc.tile_pool(name="crd", bufs=2))
        gp = st.enter_context(tc.tile_pool(name="gath", bufs=2))
        wp = st.enter_context(tc.tile_pool(name="wrp", bufs=2))

        for r in range(RH):
            # v0 init: vol_sum = feat0 slab row (broadcast over d), vol_sq = its square
            vsum = volp.tile([128, C * 80], F32, tag="vsum")
            for xh in range(2):
                in_ap = bass.AP(f0s, (r * 2 + xh) * C * 80, [[0, 64], [1, C * 80]])
                nc.sync.dma_start(vsum[xh * 64:(xh + 1) * 64, :], in_ap)
            vsq = volp.tile([128, C * 80], F32, tag="vsq")
            nc.scalar.activation(vsq[:], vsum[:], ACT.Square)

            for v in range(1, V):
                ci = (v - 1) * 3
                # affine coords: q=0 -> num_x, 1 -> num_y, 2 -> den
                nx = crd.tile([128, J], F32, tag="nx")
                ny = crd.tile([128, J], F32, tag="ny")
                dn = crd.tile([128, J], F32, tag="dn")
                for q, t in ((0, nx), (1, ny), (2, dn)):
                    nc.vector.tensor_scalar(
                        t[:], xg[:], ca[:, ci + q:ci + q + 1],
                        cb[:, (ci + q) * RH + r:(ci + q) * RH + r + 1],
                        op0=OP.mult, op1=OP.add)
                rd = crd.tile([128, J], F32, tag="rd")
                nc.vector.reciprocal(rd[:], dn[:])
                px = crd.tile([128, J], F32, tag="px")
                nc.vector.tensor_tensor(px[:], nx[:], rd[:], op=OP.mult)
                py = crd.tile([128, J], F32, tag="py")
                nc.vector.tensor_tensor(py[:], ny[:], rd[:], op=OP.mult)

                # floor via round-to-nearest(+-2^23) then correct with is_gt
                x0 = crd.tile([128, J], F32, tag="x0")
                y0 = crd.tile([128, J], F32, tag="y0")
                gt = crd.tile([128, J], F32, tag="gt")
                for pp, ff in ((px, x0), (py, y0)):
                    nc.vector.tensor_scalar(ff[:], pp[:], 8388608.0, 8388608.0,
                                            op0=OP.add, op1=OP.subtract)
                    nc.vector.tensor_tensor(gt[:], ff[:], pp[:], op=OP.is_gt)
                    nc.vector.tensor_tensor(ff[:], ff[:], gt[:], op=OP.subtract)
                wx = crd.tile([128, J], F32, tag="wx")
                nc.vector.tensor_tensor(wx[:], px[:], x0[:], op=OP.subtract)
                wy = crd.tile([128, J], F32, tag="wy")
                nc.vector.tensor_tensor(wy[:], py[:], y0[:], op=OP.subtract)

                # per-corner validity: clamp-equality
                cl = crd.tile([128, J], F32, tag="cl")
                vx0 = crd.tile([128, J], F32, tag="vx0")
                vx1 = crd.tile([128, J], F32, tag="vx1")
                vy0 = crd.tile([128, J], F32, tag="vy0")
                vy1 = crd.tile([128, J], F32, tag="vy1")
                for src_t, lo, hi, dst in (
                        (x0, 0.0, W - 1.0, vx0), (x0, -1.0, W - 2.0, vx1),
                        (y0, 0.0, H - 1.0, vy0), (y0, -1.0, H - 2.0, vy1)):
                    nc.vector.tensor_scalar(cl[:], src_t[:], lo, hi,
                                            op0=OP.max, op1=OP.min)
                    nc.vector.tensor_tensor(dst[:], cl[:], src_t[:],
                                            op=OP.is_equal)

                # corner weights (a0,a1)x(b0,b1) into w4[p, j, corner]
                a0 = crd.tile([128, J], F32, tag="a0")
                nc.vector.tensor_scalar(a0[:], wx[:], -1.0, 1.0,
                                        op0=OP.mult, op1=OP.add)
                nc.vector.tensor_tensor(a0[:], a0[:], vx0[:], op=OP.mult)
                a1 = crd.tile([128, J], F32, tag="a1")
                nc.vector.tensor_tensor(a1[:], wx[:], vx1[:], op=OP.mult)
                b0 = crd.tile([128, J], F32, tag="b0")
                nc.vector.tensor_scalar(b0[:], wy[:], -1.0, 1.0,
                                        op0=OP.mult, op1=OP.add)
                nc.vector.tensor_tensor(b0[:], b0[:], vy0[:], op=OP.mult)
                b1 = crd.tile([128, J], F32, tag="b1")
                nc.vector.tensor_tensor(b1[:], wy[:], vy1[:], op=OP.mult)
                w4 = crd.tile([128, J * 4], F32, tag="w4")
                w4r = w4[:].rearrange("p (j k) -> p j k", k=4)
                nc.vector.tensor_tensor(w4r[:, :, 0], a0[:], b0[:], op=OP.mult)
                nc.vector.tensor_tensor(w4r[:, :, 1], a0[:], b1[:], op=OP.mult)
                nc.vector.tensor_tensor(w4r[:, :, 2], a1[:], b0[:], op=OP.mult)
                nc.vector.tensor_tensor(w4r[:, :, 3], a1[:], b1[:], op=OP.mult)

                # fetch row index e = y0*W + x0 + (W+2), clamped
                ef = crd.tile([128, J], F32, tag="ef")
                nc.vector.tensor_scalar(ef[:], y0[:], float(W), float(W + 1),
                                        op0=OP.mult, op1=OP.add)
                nc.vector.tensor_tensor(ef[:], ef[:], x0[:], op=OP.add)
                nc.vector.tensor_scalar(ef[:], ef[:], 0.0, EMAX,
                                        op0=OP.max, op1=OP.min)
                ei = crd.tile([128, J], I16, tag="ei")
                nc.vector.tensor_copy(ei[:], ef[:])

                # regroup into gather-index layout + replicate to 128 partitions
                if sub == "coords":
                    continue
                idxt = crd.tile([128, 8 * J], I16, tag="idxt")
                for g in range(8):
                    nc.sync.dma_start(idxt[0:16, g::8],
                                      ei[g * 16:(g + 1) * 16, :])
                nc.sync.dma_start(idxt[16:32, :], idxt[0:16, :])
                nc.sync.dma_start(idxt[32:64, :], idxt[0:32, :])
                nc.sync.dma_start(idxt[64:128, :], idxt[0:64, :])

                g_t = gp.tile([128, J * 128], F32, tag="g")
                tab_ap = bass.AP(tabs[v - 1], 0, [[64, NENT - 1], [1, 128]])
                g_view = g_t[:].rearrange("p (j e) -> p j e", e=128)
                for s in range(J // 8):
                    gi = nc.gpsimd.dma_gather(
                        g_view[:, s * 8:(s + 1) * 8, :], tab_ap,
                        idxt[:, s * 64:(s + 1) * 64], 1024, 1024, 128,
                        elem_step=64)
                    tile.add_dep_helper(gi.ins, tc._libi.ins, sync=False,
                                        reason="gather needs mlp library")

                if sub == "gather":
                    continue
                # blend corners: G *= w4 (bcast over c), reduce over corner axis
                gv = g_t[:].rearrange("p (j k c) -> p j k c", k=4, c=C)
                wb = (w4[:].rearrange("p (j k) -> p j k", k=4)
                      .unsqueeze(3).broadcast_to([128, J, 4, C]))
                nc.vector.tensor_tensor(gv, gv, wb, op=OP.mult)
                warped = wp.tile([128, C * 80], F32, tag="warped")
                red_in = gv.transpose([0, 3, 1, 2])
                red_out = warped[:].rearrange("p (c j) -> p c j", j=80)
                nc.vector.tensor_reduce(red_out, red_in,
                                        axis=mybir.AxisListType.X, op=OP.add)
                nc.vector.tensor_tensor(vsum[:], vsum[:], warped[:], op=OP.add)
                sq = wp.tile([128, C * 80], F32, tag="tmp")
                nc.scalar.activation(sq[:], warped[:], ACT.Square)
                nc.vector.tensor_tensor(vsq[:], vsq[:], sq[:], op=OP.add)

            # variance = vsq/V - (vsum/V)^2 ; write var_t[p, c, r, j]
            m = wp.tile([128, C * 80], F32, tag="tmp")
            nc.scalar.activation(m[:], vsum[:], ACT.Square, scale=1.0 / V)
            varr = wp.tile([128, C * 80], F32, tag="tmp")
            nc.vector.scalar_tensor_tensor(varr[:], vsq[:], 1.0 / V, m[:],
                                           op0=OP.mult, op1=OP.subtract)
            out_ap = bass.AP(var_t, r * VT_J,
                             [[VT_C * VT_R * VT_J, 128], [VT_R * VT_J, C],
                              [1, VT_J]])
            nc.sync.dma_start(out_ap, varr[:])


def _emit_conv(ctx, tc, nc, bandh, identh, var_t, outh, rep):
    """3x3x3 conv via banded matmuls + softmax over depth, per output row."""
    import contextlib
    with contextlib.ExitStack() as st:
        cp = st.enter_context(tc.tile_pool(name="conv", bufs=1))
        hp = st.enter_context(tc.tile_pool(name="halo", bufs=1))
        pp = st.enter_context(tc.tile_pool(name="cpsum", bufs=2, space="PSUM"))
        sp = st.enter_context(tc.tile_pool(name="soft", bufs=2))

        band = cp.tile([128, 9 * NQ * 64], F32)
        nc.sync.dma_start(
            band[:], bass.AP(bandh, 0, [[64, 128], [8192, 9 * NQ], [1, 64]]))
        ident = cp.tile([128, 128], F32)
        nc.sync.dma_start(ident[:], identh.ap())

        # per-chunk ring of 3 var rows, free layout (rr, 162) with x pad cols
        halos = []
        for k in range(NQ):
            hk = hp.tile([128, 3 * 162], F32, tag=f"halo{k}")
            # zero the pad columns (x=0 and x=161 of each rr)
            nc.vector.memset(
                hk[:].rearrange("p (rr x) -> p rr x", x=162)[:, :, 0:162:161],
                0.0)
            halos.append(hk)

        def load_row(k, rvar, slot):
            hk = halos[k]
            for cl in range(2):
                out_ap = (hk[cl * 64:(cl + 1) * 64, :]
                          .rearrange("p (rr x) -> p rr x", x=162)
                          [:, slot, 1:161].rearrange("p (xh j) -> p xh j", j=80))
                in_ap = bass.AP(
                    var_t, (2 * k + cl) * VT_R * VT_J + rvar * VT_J,
                    [[VT_C * VT_R * VT_J, 64],
                     [64 * VT_C * VT_R * VT_J, 2], [1, VT_J]])
                nc.sync.dma_start(out_ap, in_ap)

        for k in range(NQ):
            load_row(k, 0, 0)
            load_row(k, 1, 1)

        for ro in range(1, ROWS + 1):
            for k in range(NQ):
                load_row(k, ro + 1, (ro + 1) % 3)
            cost = pp.tile([64, W], F32, tag="cost")
            first = True
            for dy in range(3):
                slot = (ro + dy - 1) % 3
                for dx in range(3):
                    t = dy * 3 + dx
                    for k in range(NQ):
                        rhs = halos[k][:, slot * 162 + dx: slot * 162 + dx + W]
                        lhsT = band[:, (t * NQ + k) * 64:(t * NQ + k + 1) * 64]
                        last = (dy == 2 and dx == 2 and k == NQ - 1)
                        nc.tensor.matmul(cost[:], lhsT, rhs,
                                         start=first, stop=last)
                        first = False
            cs = sp.tile([64, W], F32, tag="cs")
            nc.scalar.copy(cs[:], cost[:])
            # transpose to [x, d] in two partition groups, then softmax over d
            for xi, (xa, xb) in enumerate(((0, 128), (128, 160))):
                n = xb - xa
                pt = pp.tile([128, 64], F32, tag="pt")
                nc.tensor.transpose(pt[:n, :], cs[:, xa:xb], ident[0:64, 0:64])
                ct = sp.tile([128, 64], F32, tag="ct")
                nc.vector.tensor_copy(ct[:n, :], pt[:n, :])
                mx = sp.tile([128, 1], F32, tag="mx")
                nc.vector.tensor_reduce(mx[:n, :], ct[:n, :],
                                        axis=mybir.AxisListType.X, op=OP.max)
                nc.vector.tensor_scalar(mx[:n, :], mx[:n, :], -1.0, None,
                                        op0=OP.mult)
                ex = sp.tile([128, 64], F32, tag="ex")
                se = sp.tile([128, 1], F32, tag="se")
                nc.scalar.activation(ex[:n, :], ct[:n, :], ACT.Exp,
                                     bias=mx[:n, :], accum_out=se[:n, :])
                nc.vector.reciprocal(se[:n, :], se[:n, :])
                nc.vector.tensor_scalar(ex[:n, :], ex[:n, :], se[:n, :], None,
                                        op0=OP.mult)
                out_ap = bass.AP(outh, (ro - 1) * W * D + xa * D,
                                 [[D, n], [1, D]])
                nc.sync.dma_start(out_ap, ex[:n, :])


def _get_runner(nrep=1):
    if nrep in _cache:
        return _cache[nrep]
    import jax
    from jax.sharding import Mesh, PartitionSpec
    from jax.experimental.shard_map import shard_map
    from concourse.bass2jax import (_bass_exec_p, install_neuronx_cc_hook,
                                    partition_id_tensor)

    nc = _build_program(nrep)
    install_neuronx_cc_hook()
    partition_name = (nc.partition_id_tensor.name
                      if nc.partition_id_tensor else None)
    in_names, out_names, out_avals, zero_outs = [], [], [], []
    for alloc in nc.m.functions[0].allocations:
        if not isinstance(alloc, mybir.MemoryLocationSet):
            continue
        name = alloc.memorylocations[0].name
        if alloc.kind == "ExternalInput":
            if name != partition_name:
                in_names.append(name)
        elif alloc.kind == "ExternalOutput":
            shape = tuple(alloc.tensor_shape)
            dtype = mybir.dt.np(alloc.dtype)
            out_names.append(name)
            out_avals.append(jax.core.ShapedArray(shape, dtype))
            zero_outs.append(np.zeros(shape, dtype))
    n_params, n_outs = len(in_names), len(out_avals)
    all_in = list(in_names) + list(out_names) + (
        [partition_name] if partition_name else [])

    def _body(*args):
        operands = list(args)
        if partition_name is not None:
            operands.append(partition_id_tensor())
        outs = _bass_exec_p.bind(
            *operands, out_avals=tuple(out_avals), in_names=tuple(all_in),
            out_names=tuple(out_names), lowering_input_output_aliases=(),
            sim_require_finite=True, sim_require_nnan=True, nc=nc)
        return tuple(outs)

    devices = jax.devices()[:NCORES]
    mesh = Mesh(np.asarray(devices), ("core",))
    in_specs = (PartitionSpec("core"),) * (n_params + n_outs)
    out_specs = (PartitionSpec("core"),) * n_outs
    donate = tuple(range(n_params, n_params + n_outs))
    sharded = jax.jit(
        shard_map(_body, mesh=mesh, in_specs=in_specs, out_specs=out_specs,
                  check_rep=False),
        donate_argnums=donate, keep_unused=True)

    def run(in_maps):
        per_core = [[np.asarray(m[n]) for n in in_names] for m in in_maps]
        concat_in = [
            np.concatenate([per_core[c][i] for c in range(NCORES)], axis=0)
            for i in range(n_params)]
        concat_zeros = [
            np.zeros((NCORES * z.shape[0], *z.shape[1:]), z.dtype)
            for z in zero_outs]
        out_arrs = sharded(*concat_in, *concat_zeros)
        jax.block_until_ready(out_arrs)
        return [{n: np.asarray(out_arrs[i]).reshape(
                    NCORES, *out_avals[i].shape)[c]
                 for i, n in enumerate(out_names)} for c in range(NCORES)]

    _cache[nrep] = run
    return run


def _host_prep(feat0, feat1, feat2, feat3, feat4, proj_matrices, depth_values,
               conv_w):
    feats = [np.asarray(f, np.float32) for f in
             (feat0, feat1, feat2, feat3, feat4)]
    projs = np.asarray(proj_matrices, np.float32)
    depth = np.asarray(depth_values, np.float32)[0]          # [D]
    w3 = np.asarray(conv_w, np.float32)[0]                   # [C,3,3,3]

    def fuse(p):  # p [2,4,4]
        out = p[0].copy()
        out[:3, :4] = p[1, :3, :3] @ p[0, :3, :4]
        return out

    ref = fuse(projs[0, 0])
    ref_inv = np.linalg.inv(ref)
    Rs, ts = [], []
    for v in range(1, V):
        P = fuse(projs[0, v]) @ ref_inv
        Rs.append(P[:3, :3])
        ts.append(P[:3, 3])

    # zip tables: row e+1 = [Fp_flat[e], Fp_flat[e+W]] over zero-row-padded
    # pixel-major features; rows 0 and NENT-2, NENT-1 are zeros.
    tables = []
    for v in range(1, V):
        fp = np.zeros((H + 2, W, C), np.float32)
        fp[1:H + 1] = feats[v][0].transpose(1, 2, 0)
        flat = fp.reshape(-1, C)
        t = np.zeros((NENT, 64), np.float32)
        ne = (H + 1) * W
        t[1:ne + 1, :32] = flat[0:ne]
        t[1:ne + 1, 32:] = flat[W:ne + W]
        tables.append(t)

    # per-core inputs
    dgrid = np.arange(128) % 64                              # d = p % 64
    xh = np.arange(128) // 64
    xgv = (xh[:, None] * 80 + np.arange(J)[None, :]).astype(np.float32)

    band = np.zeros((9, NQ, 128, 64), np.float32)
    d_ = np.arange(64)
    dout = np.arange(64)
    dz = d_[:, None] - dout[None, :] + 1                     # [d', dout]
    msk = (dz >= 0) & (dz < 3)
    dzc = np.clip(dz, 0, 2)
    for dy in range(3):
        for dx in range(3):
            for k in range(NQ):
                for cl in range(2):
                    c = 2 * k + cl
                    blk = np.where(msk, w3[c, dzc, dy, dx], 0.0)
                    band[dy * 3 + dx, k, cl * 64:(cl + 1) * 64, :] = blk
    band = band.reshape(9 * NQ, 128, 64)

    ident = np.eye(128, dtype=np.float32)

    in_maps = []
    f0p = np.zeros((H + 2, C, W), np.float32)
    f0p[1:H + 1] = feats[0][0].transpose(1, 0, 2)            # [H,C,W] padded
    for core in range(NCORES):
        base = core * ROWS
        ys = np.arange(base - 1, base + ROWS + 1)            # RH global rows
        f0slab = f0p[ys + 1].reshape(RH, C, 2, 80).transpose(0, 2, 1, 3).copy()
        ca = np.zeros((128, 12), np.float32)
        cbm = np.zeros((128, 12 * RH), np.float32)
        for v in range(1, V):
            R, t = Rs[v - 1], ts[v - 1]
            for q in range(3):
                ca[:, (v - 1) * 3 + q] = R[q, 0] * depth[dgrid]
                for ri, y in enumerate(ys):
                    val = (R[q, 1] * y + R[q, 2]) * depth[dgrid] + t[q]
                    cbm[:, ((v - 1) * 3 + q) * RH + ri] = val
        # invalid halo rows (outside image): force py huge -> all weights 0
        for ri, y in enumerate(ys):
            if y < 0 or y >= H:
                for v in range(1, V):
                    cbm[:, ((v - 1) * 3 + 1) * RH + ri] = 1e9
        m = {f"tab{v}": tables[v - 1] for v in range(1, V)}
        m.update(f0slab=f0slab, coefa=ca, coefb=cbm, xg=xgv,
                 band=band, ident=ident)
        in_maps.append(m)
    return in_maps


def kernel(feat0, feat1, feat2, feat3, feat4, proj_matrices, depth_values,
           num_depth=None, conv_w=None, conv_b=None, **_):
    in_maps = _host_prep(feat0, feat1, feat2, feat3, feat4, proj_matrices,
                         depth_values, conv_w)
    run = _get_runner(1)
    res = run(in_maps)
    out = np.zeros((B, D, H, W), np.float32)
    for core in range(NCORES):
        o = res[core]["out"]                                 # [ROWS, W, D]
        out[0, :, core * ROWS:(core + 1) * ROWS, :] = o.transpose(2, 0, 1)
    return out



# revision 12
# speedup vs baseline: 8.0628x; 8.0628x over previous
"""DepthNet cost-volume kernel for 8 Trainium2 NeuronCores.

Strategy: shard output rows (H=128 -> 16 rows/core, +1 halo row each side).
Host precomputes, per (core,row,view,depth,x): the bilinear corner weights
(fp16) and the gather index into a per-view "quad-zip" table whose 256B rows
hold all four bilinear corners (fp16, order (c,k)).  Device work per row:
one 10240-sample dma_gather per view, a 3-op fp16 corner blend on DVE,
fp16 vsum / fp32 vsq accumulation, variance, then (fused, no barrier) the
3x3x3 conv as 144 fp16 PSUM-accumulated banded matmuls per output row and
a per-pixel softmax over depth.  Variance roundtrips DRAM via tile-pool
DRAM tiles so the Tile scheduler tracks deps and overlaps conv with the
gather/blend pipeline.  No cross-core communication.
"""
import numpy as np
import concourse.bass as bass
import concourse.tile as tile
from concourse import bacc, mybir
from concourse import library_config

F32 = mybir.dt.float32
F16 = mybir.dt.float16
I16 = mybir.dt.int16
OP = mybir.AluOpType
ACT = mybir.ActivationFunctionType

B, C, H, W, D, V = 1, 32, 128, 160, 64, 5
NCORES = 8
ROWS = H // NCORES          # output rows per core
RH = ROWS + 2               # with halo
NE = (H + 1) * W            # pixel entries in zip table
NENT = NE + 3               # + front pad + back pads
J = 80                      # x positions per half-row
NID = 128 * J               # samples per (row, view) = D * W
NQ = 16                     # (c-pair, d') contraction chunks
XPAD = 162                  # halo row width with 1-col pad each side

_cache = {}


def _build_program(nrep=1):
    import os
    scratch = int(os.environ.get("K_SCRATCH", "16384"))
    nc = bacc.Bacc("TRN2", target_bir_lowering=False, debug=False,
                   num_devices=NCORES, num_swdge_queues=4,
                   dynamic_dma_scratch_size=scratch)
    tabs = [nc.dram_tensor(f"tab{v}", [NENT, 128], F16, kind="ExternalInput")
            for v in range(1, V)]
    idxh = nc.dram_tensor("idxa", [RH, 128, 4 * 640], I16, kind="ExternalInput")
    w4h = nc.dram_tensor("w4a", [RH, 128, 4 * 320], F16, kind="ExternalInput")
    f0h = nc.dram_tensor("f0slab", [RH, 2, C * J], F16, kind="ExternalInput")
    bandh = nc.dram_tensor("band", [9 * NQ, 128, 64], F16, kind="ExternalInput")
    identh = nc.dram_tensor("ident", [64, 64], F32, kind="ExternalInput")
    outh = nc.dram_tensor("out", [ROWS, W, D], F32, kind="ExternalOutput")

    with tile.TileContext(nc) as tc:
        _emit(tc, nc, tabs, idxh, w4h, f0h, bandh, identh, outh, nrep)
    # SWDGE queue must be consistent with Tile's DMASW sem rotation, which
    # walks pool DMAs in final scheduled order: sem k pairs with queue k%4.
    cnt = 0
    for blk in nc.m.functions[0].blocks:
        for inst in blk.instructions:
            if isinstance(inst, mybir.InstDMAGatherAnt):
                inst.queue_num = cnt % 4
                cnt += 1
    nc.compile()
    return nc


def _emit(tc, nc, tabs, idxh, w4h, f0h, bandh, identh, outh, nrep):
    import contextlib
    with contextlib.ExitStack() as ctx:
        const_p = ctx.enter_context(tc.tile_pool(name="const", bufs=1))
        libi = nc.gpsimd.load_library(library_config.mlp)
        tc._libi = libi

        band = const_p.tile([128, 9 * NQ * 64], F16)
        nc.sync.dma_start(
            band[:], bass.AP(bandh, 0, [[64, 128], [8192, 9 * NQ], [1, 64]]))
        ident = const_p.tile([64, 64], F32)
        nc.sync.dma_start(ident[:], identh.ap())

        # persistent conv input ring: [p=(cl,d'), (k, slot, x+pad)]
        halo = const_p.tile([128, NQ * 3 * XPAD], F16)
        nc.vector.memset(
            halo[:].rearrange("p (k s x) -> p k s x", s=3, x=XPAD)
            [:, :, :, 0:XPAD:XPAD - 1], 0.0)

        for rep in range(nrep):
            if rep > 0:
                tc.strict_bb_all_engine_barrier()
            _emit_rep(ctx, tc, nc, tabs, idxh, w4h, f0h, band, ident, halo,
                      outh, rep)


def _emit_rep(ctx, tc, nc, tabs, idxh, w4h, f0h, band, ident, halo, outh, rep):
    import contextlib
    import os
    sub = os.environ.get("K_SUB", "full")  # gather | blend | main | full
    vsq_eng = nc.gpsimd if os.environ.get("K_VSQ", "dve") == "pool" else nc.vector
    with contextlib.ExitStack() as st:
        ip = st.enter_context(tc.tile_pool(name="idx", bufs=2))
        wp = st.enter_context(tc.tile_pool(name="wgt", bufs=2))
        vp = st.enter_context(tc.tile_pool(name="acc", bufs=2))
        gp = st.enter_context(tc.tile_pool(name="gath", bufs=2))
        bp = st.enter_context(tc.tile_pool(name="blnd", bufs=2))
        varp = st.enter_context(tc.tile_pool(name="varp", bufs=2))
        dvp = st.enter_context(tc.tile_pool(name="vdram", bufs=1, space="DRAM"))
        pp = st.enter_context(tc.tile_pool(name="cpsum", bufs=2, space="PSUM"))
        sp = st.enter_context(tc.tile_pool(name="soft", bufs=2))

        vrows = [dvp.tile([C, D, W], F16, tag=f"vr{r}", name=f"vr{r}_{rep}")
                 for r in range(RH)]

        for r in range(RH):
            idxt = ip.tile([128, 4 * 640], I16, tag="idxt")
            nc.sync.dma_start(
                idxt[:], bass.AP(idxh, r * 128 * 2560, [[2560, 128], [1, 2560]]))
            w4t = wp.tile([128, 4 * 320], F16, tag="w4")
            nc.sync.dma_start(
                w4t[:], bass.AP(w4h, r * 128 * 1280, [[1280, 128], [1, 1280]]))
            # vsum init = feat0 row (broadcast over d partitions), (c,j) order
            vsum = vp.tile([128, C * J], F16, tag="vsum")
            for xh in range(2):
                in_ap = bass.AP(f0h, (r * 2 + xh) * C * J, [[0, 64], [1, C * J]])
                nc.sync.dma_start(vsum[xh * 64:(xh + 1) * 64, :], in_ap)
            vsq = vp.tile([128, C * J], F32, tag="vsq")
            nc.scalar.activation(vsq[:], vsum[:], ACT.Square)

            gch = int(os.environ.get("K_GCH", "1024"))  # idxs per gather
            for v in range(1, V):
                g_t = gp.tile([128, J * 128], F16, tag="g")
                g_view = g_t[:].rearrange("p (j e) -> p j e", e=128)
                tab_ap = bass.AP(tabs[v - 1], 0, [[128, NENT - 1], [1, 128]])
                js = gch // 128            # j-slots per gather chunk
                for s in range(J // js):
                    gi = nc.gpsimd.dma_gather(
                        g_view[:, s * js:(s + 1) * js, :], tab_ap,
                        idxt[:, (v - 1) * 640 + s * (gch // 16):
                             (v - 1) * 640 + (s + 1) * (gch // 16)],
                        gch, gch, 128)
                    tile.add_dep_helper(gi.ins, tc._libi.ins, sync=False,
                                        reason="gather needs mlp library")
                if sub == "gather":
                    continue
                # blend: gw = g * w4 (w4 bcast over c, k innermost)
                gvr = g_t[:].rearrange("p (j c k) -> p j c k", c=C, k=4)
                w4b = (w4t[:, (v - 1) * 320:v * 320]
                       .rearrange("p (j k) -> p j k", k=4)
                       .unsqueeze(2).broadcast_to([128, J, C, 4]))
                nc.vector.tensor_tensor(gvr, gvr, w4b, op=OP.mult)
                # pair-add in place into the k=0:2 slots of the gather tile
                s1r = gvr[:, :, :, 0:2]
                nc.vector.tensor_tensor(s1r, gvr[:, :, :, 0:2],
                                        gvr[:, :, :, 2:4], op=OP.add)
                # final pair-add writes warped in (c, j)-major physical order
                warped = bp.tile([128, C * J], F16, tag="warp")
                wv = warped[:].rearrange("p (c j) -> p j c", j=J)
                nc.vector.tensor_tensor(wv, s1r[:, :, :, 0], s1r[:, :, :, 1],
                                        op=OP.add)
                nc.vector.tensor_tensor(vsum[:], vsum[:], warped[:], op=OP.add)
                sq = bp.tile([128, C * J], F32, tag="sq")
                nc.scalar.activation(sq[:], warped[:], ACT.Square)
                vsq_eng.tensor_tensor(vsq[:], vsq[:], sq[:], op=OP.add)

            if sub == "gather":
                continue
            # variance = vsq/V - (vsum/V)^2, fp16 out
            m = bp.tile([128, C * J], F32, tag="msq", bufs=1)
            nc.scalar.activation(m[:], vsum[:], ACT.Square, scale=1.0 / V)
            var16 = varp.tile([128, C * J], F16, tag="var")
            nc.vector.scalar_tensor_tensor(var16[:], vsq[:], 1.0 / V, m[:],
                                           op0=OP.mult, op1=OP.subtract)
            if sub in ("blend",):
                continue
            # var row -> DRAM [c, d, x], one DMA per xh half
            vr = vrows[r]
            for xh in range(2):
                dst = vr.transpose([1, 0, 2])[:, :, xh * J:(xh + 1) * J]
                src = (var16[xh * 64:(xh + 1) * 64, :]
                       .rearrange("p (c j) -> p c j", j=J))
                nc.sync.dma_start(dst, src)
            if sub == "main":
                continue
            # conv input load: var row r -> halo ring slot r%3, per cl half
            hv = halo[:].rearrange("p (k s x) -> p k s x", s=3, x=XPAD)
            for cl in range(2):
                dst = hv[cl * 64:(cl + 1) * 64, :, r % 3, 1:1 + W]
                src = vr[cl::2, :, :].transpose([1, 0, 2])
                nc.sync.dma_start(dst, src)

            if r < 2:
                continue
            ro = r - 1          # output row (1..16), needs var rows ro-1..ro+1
            cost = pp.tile([64, W], F32, tag="cost")
            first = True
            for dy in range(3):
                slot = (ro + dy - 1) % 3
                for dx in range(3):
                    t = dy * 3 + dx
                    for k in range(NQ):
                        rhs = hv[:, k, slot, dx:dx + W]
                        lhsT = band[:, (t * NQ + k) * 64:(t * NQ + k + 1) * 64]
                        last = (dy == 2 and dx == 2 and k == NQ - 1)
                        nc.tensor.matmul(cost[:], lhsT, rhs,
                                         start=first, stop=last)
                        first = False
            cs = sp.tile([64, W], F32, tag="cs")
            nc.scalar.copy(cs[:], cost[:])
            # transpose to [x, d] in two partition groups, softmax over d
            for xa, xb in ((0, 128), (128, 160)):
                n = xb - xa
                pt = pp.tile([128, 64], F32, tag="pt")
                nc.tensor.transpose(pt[:n, :], cs[:, xa:xb], ident[:])
                ct = sp.tile([128, 64], F32, tag="ct")
                nc.scalar.copy(ct[:n, :], pt[:n, :])
                mx = sp.tile([128, 1], F32, tag="mx")
                nc.vector.tensor_reduce(mx[:n, :], ct[:n, :],
                                        axis=mybir.AxisListType.X, op=OP.max)
                nc.vector.tensor_scalar(mx[:n, :], mx[:n, :], -1.0, None,
                                        op0=OP.mult)
                ex = sp.tile([128, 64], F32, tag="ex")
                se = sp.tile([128, 1], F32, tag="se")
                nc.scalar.activation(ex[:n, :], ct[:n, :], ACT.Exp,
                                     bias=mx[:n, :], accum_out=se[:n, :])
                nc.vector.reciprocal(se[:n, :], se[:n, :])
                nc.vector.tensor_scalar(ex[:n, :], ex[:n, :], se[:n, :], None,
                                        op0=OP.mult)
                out_ap = bass.AP(outh, (ro - 1) * W * D + xa * D,
                                 [[D, n], [1, D]])
                nc.sync.dma_start(out_ap, ex[:n, :])


def _get_runner(nrep=1):
    if nrep in _cache:
        return _cache[nrep]
    import jax
    from jax.sharding import Mesh, NamedSharding, PartitionSpec
    from jax.experimental.shard_map import shard_map
    from concourse.bass2jax import (_bass_exec_p, install_neuronx_cc_hook,
                                    partition_id_tensor)

    nc = _build_program(nrep)
    install_neuronx_cc_hook()
    partition_name = (nc.partition_id_tensor.name
                      if nc.partition_id_tensor else None)
    in_names, out_names, out_avals, zero_outs = [], [], [], []
    for alloc in nc.m.functions[0].allocations:
        if not isinstance(alloc, mybir.MemoryLocationSet):
            continue
        name = alloc.memorylocations[0].name
        if alloc.kind == "ExternalInput":
            if name != partition_name:
                in_names.append(name)
        elif alloc.kind == "ExternalOutput":
            shape = tuple(alloc.tensor_shape)
            dtype = mybir.dt.np(alloc.dtype)
            out_names.append(name)
            out_avals.append(jax.core.ShapedArray(shape, dtype))
            zero_outs.append(np.zeros(shape, dtype))
    n_params, n_outs = len(in_names), len(out_avals)
    all_in = list(in_names) + list(out_names) + (
        [partition_name] if partition_name else [])

    def _body(*args):
        operands = list(args)
        if partition_name is not None:
            operands.append(partition_id_tensor())
        outs = _bass_exec_p.bind(
            *operands, out_avals=tuple(out_avals), in_names=tuple(all_in),
            out_names=tuple(out_names), lowering_input_output_aliases=(),
            sim_require_finite=True, sim_require_nnan=True, nc=nc)
        return tuple(outs)

    devices = jax.devices()[:NCORES]
    mesh = Mesh(np.asarray(devices), ("core",))
    in_specs = (PartitionSpec("core"),) * (n_params + n_outs)
    out_specs = (PartitionSpec("core"),) * n_outs
    donate = tuple(range(n_params, n_params + n_outs))
    sharded = jax.jit(
        shard_map(_body, mesh=mesh, in_specs=in_specs, out_specs=out_specs,
                  check_rep=False),
        donate_argnums=donate, keep_unused=True)
    shard = NamedSharding(mesh, PartitionSpec("core"))
    dev_cache = {}

    import os
    use_devput = os.environ.get("K_DEVPUT", "0") == "1"

    def run(in_maps):
        key = id(in_maps)
        if key not in dev_cache:
            per_core = [[np.asarray(m[n]) for n in in_names] for m in in_maps]
            concat_in = [
                np.concatenate([per_core[c][i] for c in range(NCORES)], axis=0)
                for i in range(n_params)]
            if use_devput:
                concat_in = [jax.device_put(a, shard) for a in concat_in]
            dev_cache[key] = concat_in
        dev_in = dev_cache[key]
        concat_zeros = [
            np.zeros((NCORES * z.shape[0], *z.shape[1:]), z.dtype)
            for z in zero_outs]
        out_arrs = sharded(*dev_in, *concat_zeros)
        jax.block_until_ready(out_arrs)
        return [{n: np.asarray(out_arrs[i]).reshape(
                    NCORES, *out_avals[i].shape)[c]
                 for i, n in enumerate(out_names)} for c in range(NCORES)]

    _cache[nrep] = run
    return run


def _host_prep(feat0, feat1, feat2, feat3, feat4, proj_matrices, depth_values,
               conv_w):
    feats = [np.asarray(f, np.float32) for f in
             (feat0, feat1, feat2, feat3, feat4)]
    projs = np.asarray(proj_matrices, np.float32)
    depth = np.asarray(depth_values, np.float32)[0]          # [D]
    w3 = np.asarray(conv_w, np.float32)[0]                   # [C,3,3,3]

    def fuse(p):  # p [2,4,4]
        out = p[0].copy()
        out[:3, :4] = p[1, :3, :3] @ p[0, :3, :4]
        return out

    ref = fuse(projs[0, 0])
    ref_inv = np.linalg.inv(ref)
    Rs, ts = [], []
    for v in range(1, V):
        P = fuse(projs[0, v]) @ ref_inv
        Rs.append(P[:3, :3])
        ts.append(P[:3, 3])

    # quad-zip tables: row 1+e' (e' = y'*W + x over zero-row-padded image)
    # holds fp16 corners [(y',x),(y',x+1),(y'+1,x),(y'+1,x+1)] in (c,k) order.
    tables = []
    for v in range(1, V):
        fp = np.zeros(((H + 2) * W + 1, C), np.float32)
        fp[W:W + H * W] = feats[v][0].transpose(1, 2, 0).reshape(H * W, C)
        e = np.arange(NE)
        q = np.stack([fp[e], fp[e + 1], fp[e + W], fp[e + W + 1]], axis=-1)
        t = np.zeros((NENT, 128), np.float16)
        t[1:NE + 1] = q.reshape(NE, 128).astype(np.float16)
        tables.append(t)

    # coords/weights/indices for all global rows y in [-1, H]
    dgrid = (np.arange(128) % 64)
    xhg = np.arange(128) // 64
    dep = depth[dgrid]                                       # [128]
    xv = (xhg[:, None] * J + np.arange(J)[None, :]).astype(np.float32)
    yv = np.arange(-1, H + 1, dtype=np.float32)              # [H+2]
    NY = H + 2

    e_all = np.zeros((NY, 128, V - 1, J), np.int16)
    w_all = np.zeros((NY, 128, V - 1, J, 4), np.float16)
    with np.errstate(all="ignore"):
        for v in range(1, V):
            R, t = Rs[v - 1], ts[v - 1]
            # n_q [NY, 128, J]
            nx = (R[0, 0] * xv * dep[:, None])[None] \
                + ((R[0, 1] * yv[:, None] + R[0, 2]) * dep[None, :])[:, :, None] \
                + t[0]
            ny = (R[1, 0] * xv * dep[:, None])[None] \
                + ((R[1, 1] * yv[:, None] + R[1, 2]) * dep[None, :])[:, :, None] \
                + t[1]
            dn = (R[2, 0] * xv * dep[:, None])[None] \
                + ((R[2, 1] * yv[:, None] + R[2, 2]) * dep[None, :])[:, :, None] \
                + t[2]
            px = nx / dn
            py = ny / dn
            fin = np.isfinite(px) & np.isfinite(py)
            px = np.where(fin, px, 0.0)
            py = np.where(fin, py, 0.0)
            x0 = np.floor(px)
            y0 = np.floor(py)
            wx = px - x0
            wy = py - y0
            vx0 = (x0 >= 0) & (x0 <= W - 1)
            vx1 = (x0 >= -1) & (x0 <= W - 2)
            vy0 = (y0 >= 0) & (y0 <= H - 1)
            vy1 = (y0 >= -1) & (y0 <= H - 2)
            a0 = (1 - wx) * vx0 * fin
            a1 = wx * vx1 * fin
            b0 = (1 - wy) * vy0
            b1 = wy * vy1
            # rows outside the image contribute nothing (ref warps from
            # valid ref pixels only per-core; halo rows y<0 or y>=H excluded)
            yok = (yv >= 0) & (yv <= H - 1)
            a0 *= yok[:, None, None]
            a1 *= yok[:, None, None]
            w4 = np.stack([a0 * b0, a1 * b0, a0 * b1, a1 * b1], axis=-1)
            x0c = np.clip(x0, -1, W - 1)
            y0c = np.clip(y0, -1, H - 1)
            e = 1 + (y0c + 1) * W + x0c
            e = np.clip(e, 0, NENT - 2)
            e_all[:, :, v - 1, :] = e.astype(np.int16)
            w_all[:, :, v - 1, :, :] = w4.astype(np.float16)

    band = np.zeros((9, NQ, 128, 64), np.float32)
    d_ = np.arange(64)
    dz = d_[:, None] - d_[None, :] + 1                       # [d', dout]
    msk = (dz >= 0) & (dz < 3)
    dzc = np.clip(dz, 0, 2)
    for dy in range(3):
        for dx in range(3):
            for k in range(NQ):
                for cl in range(2):
                    c = 2 * k + cl
                    blk = np.where(msk, w3[c, dzc, dy, dx], 0.0)
                    band[dy * 3 + dx, k, cl * 64:(cl + 1) * 64, :] = blk
    band = band.reshape(9 * NQ, 128, 64).astype(np.float16)
    ident = np.eye(64, dtype=np.float32)

    # feat0 slab padded, (c,j) order per (row, xh)
    f0p = np.zeros((H + 2, C, W), np.float32)
    f0p[1:H + 1] = feats[0][0].transpose(1, 0, 2)

    in_maps = []
    for core in range(NCORES):
        base = core * ROWS
        rs = np.arange(base, base + RH)                      # y = base-1+r
        e_r = e_all[rs]                                      # [RH,128,4,80]
        w_r = w_all[rs]                                      # [RH,128,4,80,4]
        # idx layout: sample i = p + 128*j -> idx[q, 8j+g], q=p%16, g=p//16
        tmp = e_r.reshape(RH, 8, 16, 4, J)
        idx16 = tmp.transpose(0, 2, 3, 4, 1).reshape(RH, 16, 4 * 640)
        idxa = np.broadcast_to(idx16[:, None], (RH, 8, 16, 4 * 640))
        idxa = np.ascontiguousarray(idxa.reshape(RH, 128, 4 * 640))
        w4a = np.ascontiguousarray(w_r.reshape(RH, 128, 4 * 320))
        f0slab = (f0p[rs].reshape(RH, C, 2, J).transpose(0, 2, 1, 3)
                  .reshape(RH, 2, C * J).astype(np.float16))
        m = {f"tab{v}": tables[v - 1] for v in range(1, V)}
        m.update(idxa=idxa, w4a=w4a, f0slab=f0slab, band=band, ident=ident)
        in_maps.append(m)
    return in_maps


def kernel(feat0, feat1, feat2, feat3, feat4, proj_matrices, depth_values,
           num_depth=None, conv_w=None, conv_b=None, **_):
    in_maps = _host_prep(feat0, feat1, feat2, feat3, feat4, proj_matrices,
                         depth_values, conv_w)
    run = _get_runner(1)
    res = run(in_maps)
    out = np.zeros((B, D, H, W), np.float32)
    for core in range(NCORES):
        o = res[core]["out"]                                 # [ROWS, W, D]
        out[0, :, core * ROWS:(core + 1) * ROWS, :] = o.transpose(2, 0, 1)
    return out


# revision 24
# speedup vs baseline: 19.3967x; 2.4057x over previous
"""DepthNet cost-volume kernel for 8 Trainium2 NeuronCores.

Strategy: shard output rows (H=128 -> 16 rows/core, +1 halo row each side).
Host precomputes, per (core,row,view,depth,x): the bilinear corner weights
(fp16) and the gather index into a per-view "quad-zip" table whose 256B rows
hold all four bilinear corners (fp16, order (c,k)).  Device work per row:
one 10240-sample dma_gather per view, a 3-op fp16 corner blend on DVE,
fp16 vsum / fp32 vsq accumulation, variance, then (fused, no barrier) the
3x3x3 conv as 144 fp16 PSUM-accumulated banded matmuls per output row and
a per-pixel softmax over depth.  Variance roundtrips DRAM via tile-pool
DRAM tiles so the Tile scheduler tracks deps and overlaps conv with the
gather/blend pipeline.  No cross-core communication.
"""
import numpy as np
import concourse.bass as bass
import concourse.tile as tile
from concourse import bacc, mybir
from concourse import library_config

F32 = mybir.dt.float32
F16 = mybir.dt.float16
I16 = mybir.dt.int16
OP = mybir.AluOpType
ACT = mybir.ActivationFunctionType

B, C, H, W, D, V = 1, 32, 128, 160, 64, 5
NCORES = 8
ROWS = H // NCORES          # output rows per core
RH = ROWS + 2               # with halo
NE = (H + 1) * W            # pixel entries in zip table
NENT = NE + 3               # + front pad + back pads
J = 80                      # x positions per half-row
NID = 128 * J               # samples per (row, view) = D * W
NQ = 16                     # (c-pair, d') contraction chunks
XPAD = 162                  # halo row width with 1-col pad each side

_cache = {}


def _build_program(nrep=1):
    import os
    scratch = int(os.environ.get("K_SCRATCH", "16384"))
    nc = bacc.Bacc("TRN2", target_bir_lowering=False, debug=False,
                   num_devices=NCORES, num_swdge_queues=4,
                   dynamic_dma_scratch_size=scratch)
    tabs = [nc.dram_tensor(f"tab{v}", [NENT, 128], F16, kind="ExternalInput")
            for v in range(1, V)]
    idxh = nc.dram_tensor("idxa", [RH, 128, 4 * 640], I16, kind="ExternalInput")
    w4h = nc.dram_tensor("w4a", [RH, 128, 4 * 320], F16, kind="ExternalInput")
    f0h = nc.dram_tensor("f0slab", [RH, 2, C * J], F16, kind="ExternalInput")
    bandh = nc.dram_tensor("band", [9 * NQ, 128, 64], F16, kind="ExternalInput")
    identh = nc.dram_tensor("ident", [64, 64], F32, kind="ExternalInput")
    outh = nc.dram_tensor("out", [ROWS, W, D], F32, kind="ExternalOutput")

    with tile.TileContext(nc) as tc:
        _emit(tc, nc, tabs, idxh, w4h, f0h, bandh, identh, outh, nrep)
    # SWDGE queue must be consistent with Tile's DMASW sem rotation, which
    # walks pool DMAs in final scheduled order: sem k pairs with queue k%4.
    cnt = 0
    for blk in nc.m.functions[0].blocks:
        for inst in blk.instructions:
            if isinstance(inst, mybir.InstDMAGatherAnt):
                inst.queue_num = cnt % 4
                cnt += 1
    nc.compile()
    return nc


def _emit(tc, nc, tabs, idxh, w4h, f0h, bandh, identh, outh, nrep):
    import contextlib
    with contextlib.ExitStack() as ctx:
        const_p = ctx.enter_context(tc.tile_pool(name="const", bufs=1))
        libi = nc.gpsimd.load_library(library_config.mlp)
        tc._libi = libi

        band = const_p.tile([128, 9 * NQ * 64], F16)
        nc.sync.dma_start(
            band[:], bass.AP(bandh, 0, [[64, 128], [8192, 9 * NQ], [1, 64]]))
        ident = const_p.tile([64, 64], F32)
        nc.sync.dma_start(ident[:], identh.ap())

        for rep in range(nrep):
            if rep > 0:
                tc.strict_bb_all_engine_barrier()
            _emit_rep(ctx, tc, nc, tabs, idxh, w4h, f0h, band, ident,
                      outh, rep)


def _emit_rep(ctx, tc, nc, tabs, idxh, w4h, f0h, band, ident, outh, rep):
    import contextlib
    import os
    sub = os.environ.get("K_SUB", "full")  # gather | blend | main | full
    vsq_eng = nc.gpsimd if os.environ.get("K_VSQ", "dve") == "pool" else nc.vector
    with contextlib.ExitStack() as st:
        ip = st.enter_context(tc.tile_pool(name="idx", bufs=2))
        wp = st.enter_context(tc.tile_pool(name="wgt", bufs=2))
        vp = st.enter_context(tc.tile_pool(name="acc", bufs=2))
        gp = st.enter_context(tc.tile_pool(name="gath", bufs=2))
        bp = st.enter_context(tc.tile_pool(name="blnd", bufs=2))
        varp = st.enter_context(tc.tile_pool(name="varp", bufs=2))
        dvp = st.enter_context(tc.tile_pool(name="vdram", bufs=1, space="DRAM"))
        pp = st.enter_context(tc.tile_pool(name="cpsum", bufs=2, space="PSUM"))
        sp = st.enter_context(tc.tile_pool(name="soft", bufs=2))
        hp = st.enter_context(tc.tile_pool(name="halo", bufs=1))

        # var row DRAM layout [p=(xh,d), (c,j)] — write is one contiguous DMA
        vrows = [dvp.tile([128, C * J], F16, tag=f"vr{r}", name=f"vr{r}_{rep}")
                 for r in range(RH)]
        # conv input pair tiles: pair a holds var rows (a, a+1) as
        # [p=(cl,d'), (k, pos, x+pad)]; tag rotation gives WAR safety
        pair_tiles = {}

        for r in range(RH):
            idxt = ip.tile([128, 4 * 640], I16, tag="idxt")
            nc.sync.dma_start(
                idxt[:], bass.AP(idxh, r * 128 * 2560, [[2560, 128], [1, 2560]]))
            w4t = wp.tile([128, 4 * 320], F16, tag="w4")
            nc.sync.dma_start(
                w4t[:], bass.AP(w4h, r * 128 * 1280, [[1280, 128], [1, 1280]]))
            # vsum init = feat0 row (broadcast over d partitions), (c,j) order
            vsum = vp.tile([128, C * J], F16, tag="vsum")
            for xh in range(2):
                in_ap = bass.AP(f0h, (r * 2 + xh) * C * J, [[0, 64], [1, C * J]])
                nc.sync.dma_start(vsum[xh * 64:(xh + 1) * 64, :], in_ap)
            vsq = vp.tile([128, C * J], F32, tag="vsq")
            nc.scalar.activation(vsq[:], vsum[:], ACT.Square)

            gch = int(os.environ.get("K_GCH", "1024"))  # idxs per gather
            for v in range(1, V):
                g_t = gp.tile([128, J * 128], F16, tag="g")
                g_view = g_t[:].rearrange("p (j e) -> p j e", e=128)
                tab_ap = bass.AP(tabs[v - 1], 0, [[128, NENT - 1], [1, 128]])
                js = gch // 128            # j-slots per gather chunk
                for s in range(J // js):
                    gi = nc.gpsimd.dma_gather(
                        g_view[:, s * js:(s + 1) * js, :], tab_ap,
                        idxt[:, (v - 1) * 640 + s * (gch // 16):
                             (v - 1) * 640 + (s + 1) * (gch // 16)],
                        gch, gch, 128)
                    tile.add_dep_helper(gi.ins, tc._libi.ins, sync=False,
                                        reason="gather needs mlp library")
                if sub == "gather":
                    continue
                # blend: gw = g * w4 (w4 bcast over c, k innermost)
                gvr = g_t[:].rearrange("p (j c k) -> p j c k", c=C, k=4)
                w4b = (w4t[:, (v - 1) * 320:v * 320]
                       .rearrange("p (j k) -> p j k", k=4)
                       .unsqueeze(2).broadcast_to([128, J, C, 4]))
                nc.vector.tensor_tensor(gvr, gvr, w4b, op=OP.mult)
                # pair-add in place into the k=0:2 slots of the gather tile
                s1r = gvr[:, :, :, 0:2]
                nc.vector.tensor_tensor(s1r, gvr[:, :, :, 0:2],
                                        gvr[:, :, :, 2:4], op=OP.add)
                # final pair-add writes warped in (c, j)-major physical order
                warped = bp.tile([128, C * J], F16, tag="warp")
                wv = warped[:].rearrange("p (c j) -> p j c", j=J)
                nc.vector.tensor_tensor(wv, s1r[:, :, :, 0], s1r[:, :, :, 1],
                                        op=OP.add)
                nc.vector.tensor_tensor(vsum[:], vsum[:], warped[:], op=OP.add)
                sq = bp.tile([128, C * J], F32, tag="sq")
                nc.scalar.activation(sq[:], warped[:], ACT.Square)
                vsq_eng.tensor_tensor(vsq[:], vsq[:], sq[:], op=OP.add)

            if sub == "gather":
                continue
            # variance = vsq/V - (vsum/V)^2, fp16 out
            m = bp.tile([128, C * J], F32, tag="msq", bufs=1)
            nc.scalar.activation(m[:], vsum[:], ACT.Square, scale=1.0 / V)
            var16 = varp.tile([128, C * J], F16, tag="var")
            nc.vector.scalar_tensor_tensor(var16[:], vsq[:], 1.0 / V, m[:],
                                           op0=OP.mult, op1=OP.subtract)
            if sub in ("blend",):
                continue
            # var row -> DRAM, one fully contiguous DMA (128 x 5KB descs)
            vr = vrows[r]
            nc.sync.dma_start(vr, var16[:])
            if sub == "main":
                continue
            # conv input load: row r enters pair tile (r-1) at pos 1 and a
            # fresh pair tile (r) at pos 0; 4 DMAs (cl x xh) per destination
            def load_row(rv, pa, pos):
                pv = (pair_tiles[pa][:]
                      .rearrange("p (k r x) -> p k r x", r=2, x=XPAD))
                for cl in range(2):
                    for xh in range(2):
                        dst = pv[cl * 64:(cl + 1) * 64, :, pos,
                                 1 + xh * J:1 + (xh + 1) * J]
                        src = (vrows[rv][xh * 64:(xh + 1) * 64, :]
                               .rearrange("p (c j) -> p c j", j=J)[:, cl::2, :])
                        nc.sync.dma_start(dst, src)

            if r >= 1:
                load_row(r, r - 1, 1)
            if r <= RH - 2:
                pt_ = hp.tile([128, NQ * 2 * XPAD], F16, tag=f"ps{r % 3}",
                              name=f"pair{r}_{rep}")
                nc.vector.memset(
                    pt_[:].rearrange("p (k r x) -> p k r x", r=2, x=XPAD)
                    [:, :, :, 0:XPAD:XPAD - 1], 0.0)
                pair_tiles[r] = pt_
                load_row(r, r, 0)

            # conv+softmax for output row pair (ro, ro+1) once rows <= r done
            if r < 3 or r % 2 == 0:
                continue
            ro = r - 2          # odd: 1, 3, ..., 15
            cost = pp.tile([64, 2 * W], F32, tag="cost")
            cv = cost[:].rearrange("p (r x) -> p r x", r=2)
            first = True
            for dy in range(3):
                pv = (pair_tiles[ro + dy - 1][:]
                      .rearrange("p (k r x) -> p k r x", r=2, x=XPAD))
                for dx in range(3):
                    t = dy * 3 + dx
                    for k in range(NQ):
                        rhs = pv[:, k, :, dx:dx + W]
                        lhsT = band[:, (t * NQ + k) * 64:(t * NQ + k + 1) * 64]
                        last = (dy == 2 and dx == 2 and k == NQ - 1)
                        nc.tensor.matmul(cv, lhsT, rhs,
                                         start=first, stop=last)
                        first = False
            cs = sp.tile([64, 2 * W], F32, tag="cs")
            nc.scalar.copy(cs[:], cost[:])
            # transpose to [x, d] in 128-col chunks, softmax over d; the
            # (pair, x) columns are contiguous in the output DRAM row-major
            for ci, (xa, xb) in enumerate(((0, 128), (128, 256), (256, 320))):
                n = xb - xa
                pt = pp.tile([128, 64], F32, tag="pt")
                nc.tensor.transpose(pt[:n, :], cs[:, xa:xb], ident[:])
                ct = sp.tile([128, 64], F32, tag="ct")
                nc.scalar.copy(ct[:n, :], pt[:n, :])
                mx = sp.tile([128, 1], F32, tag="mx")
                nc.vector.tensor_reduce(mx[:n, :], ct[:n, :],
                                        axis=mybir.AxisListType.X, op=OP.max)
                nc.vector.tensor_scalar(mx[:n, :], mx[:n, :], -1.0, None,
                                        op0=OP.mult)
                ex = sp.tile([128, 64], F32, tag="ex")
                se = sp.tile([128, 1], F32, tag="se")
                nc.scalar.activation(ex[:n, :], ct[:n, :], ACT.Exp,
                                     bias=mx[:n, :], accum_out=se[:n, :])
                nc.vector.reciprocal(se[:n, :], se[:n, :])
                nc.vector.tensor_scalar(ex[:n, :], ex[:n, :], se[:n, :], None,
                                        op0=OP.mult)
                out_ap = bass.AP(outh, (ro - 1) * W * D + xa * D,
                                 [[D, n], [1, D]])
                nc.sync.dma_start(out_ap, ex[:n, :])


def _get_runner(nrep=1):
    if nrep in _cache:
        return _cache[nrep]
    import jax
    from jax.sharding import Mesh, NamedSharding, PartitionSpec
    from jax.experimental.shard_map import shard_map
    from concourse.bass2jax import (_bass_exec_p, install_neuronx_cc_hook,
                                    partition_id_tensor)

    nc = _build_program(nrep)
    install_neuronx_cc_hook()
    partition_name = (nc.partition_id_tensor.name
                      if nc.partition_id_tensor else None)
    in_names, out_names, out_avals, zero_outs = [], [], [], []
    for alloc in nc.m.functions[0].allocations:
        if not isinstance(alloc, mybir.MemoryLocationSet):
            continue
        name = alloc.memorylocations[0].name
        if alloc.kind == "ExternalInput":
            if name != partition_name:
                in_names.append(name)
        elif alloc.kind == "ExternalOutput":
            shape = tuple(alloc.tensor_shape)
            dtype = mybir.dt.np(alloc.dtype)
            out_names.append(name)
            out_avals.append(jax.core.ShapedArray(shape, dtype))
            zero_outs.append(np.zeros(shape, dtype))
    n_params, n_outs = len(in_names), len(out_avals)
    all_in = list(in_names) + list(out_names) + (
        [partition_name] if partition_name else [])

    def _body(*args):
        operands = list(args)
        if partition_name is not None:
            operands.append(partition_id_tensor())
        outs = _bass_exec_p.bind(
            *operands, out_avals=tuple(out_avals), in_names=tuple(all_in),
            out_names=tuple(out_names), lowering_input_output_aliases=(),
            sim_require_finite=True, sim_require_nnan=True, nc=nc)
        return tuple(outs)

    devices = jax.devices()[:NCORES]
    mesh = Mesh(np.asarray(devices), ("core",))
    in_specs = (PartitionSpec("core"),) * (n_params + n_outs)
    out_specs = (PartitionSpec("core"),) * n_outs
    donate = tuple(range(n_params, n_params + n_outs))
    sharded = jax.jit(
        shard_map(_body, mesh=mesh, in_specs=in_specs, out_specs=out_specs,
                  check_rep=False),
        donate_argnums=donate, keep_unused=True)
    shard = NamedSharding(mesh, PartitionSpec("core"))
    dev_cache = {}

    import os
    use_devput = os.environ.get("K_DEVPUT", "0") == "1"

    def run(in_maps):
        key = id(in_maps)
        if key not in dev_cache:
            per_core = [[np.asarray(m[n]) for n in in_names] for m in in_maps]
            concat_in = [
                np.concatenate([per_core[c][i] for c in range(NCORES)], axis=0)
                for i in range(n_params)]
            if use_devput:
                concat_in = [jax.device_put(a, shard) for a in concat_in]
            dev_cache[key] = concat_in
        dev_in = dev_cache[key]
        concat_zeros = [
            np.zeros((NCORES * z.shape[0], *z.shape[1:]), z.dtype)
            for z in zero_outs]
        out_arrs = sharded(*dev_in, *concat_zeros)
        jax.block_until_ready(out_arrs)
        return [{n: np.asarray(out_arrs[i]).reshape(
                    NCORES, *out_avals[i].shape)[c]
                 for i, n in enumerate(out_names)} for c in range(NCORES)]

    _cache[nrep] = run
    return run


def _host_prep(feat0, feat1, feat2, feat3, feat4, proj_matrices, depth_values,
               conv_w):
    feats = [np.asarray(f, np.float32) for f in
             (feat0, feat1, feat2, feat3, feat4)]
    projs = np.asarray(proj_matrices, np.float32)
    depth = np.asarray(depth_values, np.float32)[0]          # [D]
    w3 = np.asarray(conv_w, np.float32)[0]                   # [C,3,3,3]

    def fuse(p):  # p [2,4,4]
        out = p[0].copy()
        out[:3, :4] = p[1, :3, :3] @ p[0, :3, :4]
        return out

    ref = fuse(projs[0, 0])
    ref_inv = np.linalg.inv(ref)
    Rs, ts = [], []
    for v in range(1, V):
        P = fuse(projs[0, v]) @ ref_inv
        Rs.append(P[:3, :3])
        ts.append(P[:3, 3])

    # quad-zip tables: row 1+e' (e' = y'*W + x over zero-row-padded image)
    # holds fp16 corners [(y',x),(y',x+1),(y'+1,x),(y'+1,x+1)] in (c,k) order.
    tables = []
    for v in range(1, V):
        fp = np.zeros(((H + 2) * W + 1, C), np.float32)
        fp[W:W + H * W] = feats[v][0].transpose(1, 2, 0).reshape(H * W, C)
        e = np.arange(NE)
        q = np.stack([fp[e], fp[e + 1], fp[e + W], fp[e + W + 1]], axis=-1)
        t = np.zeros((NENT, 128), np.float16)
        t[1:NE + 1] = q.reshape(NE, 128).astype(np.float16)
        tables.append(t)

    # coords/weights/indices for all global rows y in [-1, H]
    dgrid = (np.arange(128) % 64)
    xhg = np.arange(128) // 64
    dep = depth[dgrid]                                       # [128]
    xv = (xhg[:, None] * J + np.arange(J)[None, :]).astype(np.float32)
    yv = np.arange(-1, H + 1, dtype=np.float32)              # [H+2]
    NY = H + 2

    e_all = np.zeros((NY, 128, V - 1, J), np.int16)
    w_all = np.zeros((NY, 128, V - 1, J, 4), np.float16)
    with np.errstate(all="ignore"):
        for v in range(1, V):
            R, t = Rs[v - 1], ts[v - 1]
            # n_q [NY, 128, J]
            nx = (R[0, 0] * xv * dep[:, None])[None] \
                + ((R[0, 1] * yv[:, None] + R[0, 2]) * dep[None, :])[:, :, None] \
                + t[0]
            ny = (R[1, 0] * xv * dep[:, None])[None] \
                + ((R[1, 1] * yv[:, None] + R[1, 2]) * dep[None, :])[:, :, None] \
                + t[1]
            dn = (R[2, 0] * xv * dep[:, None])[None] \
                + ((R[2, 1] * yv[:, None] + R[2, 2]) * dep[None, :])[:, :, None] \
                + t[2]
            px = nx / dn
            py = ny / dn
            fin = np.isfinite(px) & np.isfinite(py)
            px = np.where(fin, px, 0.0)
            py = np.where(fin, py, 0.0)
            x0 = np.floor(px)
            y0 = np.floor(py)
            wx = px - x0
            wy = py - y0
            vx0 = (x0 >= 0) & (x0 <= W - 1)
            vx1 = (x0 >= -1) & (x0 <= W - 2)
            vy0 = (y0 >= 0) & (y0 <= H - 1)
            vy1 = (y0 >= -1) & (y0 <= H - 2)
            a0 = (1 - wx) * vx0 * fin
            a1 = wx * vx1 * fin
            b0 = (1 - wy) * vy0
            b1 = wy * vy1
            # rows outside the image contribute nothing (ref warps from
            # valid ref pixels only per-core; halo rows y<0 or y>=H excluded)
            yok = (yv >= 0) & (yv <= H - 1)
            a0 *= yok[:, None, None]
            a1 *= yok[:, None, None]
            w4 = np.stack([a0 * b0, a1 * b0, a0 * b1, a1 * b1], axis=-1)
            x0c = np.clip(x0, -1, W - 1)
            y0c = np.clip(y0, -1, H - 1)
            e = 1 + (y0c + 1) * W + x0c
            e = np.clip(e, 0, NENT - 2)
            e_all[:, :, v - 1, :] = e.astype(np.int16)
            w_all[:, :, v - 1, :, :] = w4.astype(np.float16)

    band = np.zeros((9, NQ, 128, 64), np.float32)
    d_ = np.arange(64)
    dz = d_[:, None] - d_[None, :] + 1                       # [d', dout]
    msk = (dz >= 0) & (dz < 3)
    dzc = np.clip(dz, 0, 2)
    for dy in range(3):
        for dx in range(3):
            for k in range(NQ):
                for cl in range(2):
                    c = 2 * k + cl
                    blk = np.where(msk, w3[c, dzc, dy, dx], 0.0)
                    band[dy * 3 + dx, k, cl * 64:(cl + 1) * 64, :] = blk
    band = band.reshape(9 * NQ, 128, 64).astype(np.float16)
    ident = np.eye(64, dtype=np.float32)

    # feat0 slab padded, (c,j) order per (row, xh)
    f0p = np.zeros((H + 2, C, W), np.float32)
    f0p[1:H + 1] = feats[0][0].transpose(1, 0, 2)

    in_maps = []
    for core in range(NCORES):
        base = core * ROWS
        rs = np.arange(base, base + RH)                      # y = base-1+r
        e_r = e_all[rs]                                      # [RH,128,4,80]
        w_r = w_all[rs]                                      # [RH,128,4,80,4]
        # idx layout: sample i = p + 128*j -> idx[q, 8j+g], q=p%16, g=p//16
        tmp = e_r.reshape(RH, 8, 16, 4, J)
        idx16 = tmp.transpose(0, 2, 3, 4, 1).reshape(RH, 16, 4 * 640)
        idxa = np.broadcast_to(idx16[:, None], (RH, 8, 16, 4 * 640))
        idxa = np.ascontiguousarray(idxa.reshape(RH, 128, 4 * 640))
        w4a = np.ascontiguousarray(w_r.reshape(RH, 128, 4 * 320))
        f0slab = (f0p[rs].reshape(RH, C, 2, J).transpose(0, 2, 1, 3)
                  .reshape(RH, 2, C * J).astype(np.float16))
        m = {f"tab{v}": tables[v - 1] for v in range(1, V)}
        m.update(idxa=idxa, w4a=w4a, f0slab=f0slab, band=band, ident=ident)
        in_maps.append(m)
    return in_maps


def kernel(feat0, feat1, feat2, feat3, feat4, proj_matrices, depth_values,
           num_depth=None, conv_w=None, conv_b=None, **_):
    in_maps = _host_prep(feat0, feat1, feat2, feat3, feat4, proj_matrices,
                         depth_values, conv_w)
    run = _get_runner(1)
    res = run(in_maps)
    out = np.zeros((B, D, H, W), np.float32)
    for core in range(NCORES):
        o = res[core]["out"]                                 # [ROWS, W, D]
        out[0, :, core * ROWS:(core + 1) * ROWS, :] = o.transpose(2, 0, 1)
    return out
